# revision 18
# baseline (speedup 1.0000x reference)
# Trainium2 Bass kernel for nn_ModelPositional (gnn_message_passing).
# v8: chain-first ordering — the collective firmware needs ~67us to boot,
# so run the RWPE chain first on the PE, land both AllGather payloads
# just as the CC comes up, and bridge the remaining gather latency with
# the stage-2 pre-phase (k0..5 held across all 8 PSUM banks).

import numpy as np

B, S, KPE, V, D = 4, 512, 16, 50265, 768
NCORES = 8
VPAD = 50272          # 8 * 6284
VC = VPAD // NCORES   # 6284 vocab columns per core
DPAD = 896            # 7 * 128 (784 true dims + bias row at 784 + zero pad)
KCH = DPAD // 128     # 7 contraction chunks
P = 128
NTOK = B * S          # 2048
MT = NTOK // P        # 16 token tiles

_CACHE = {}


def _build_nc(vc=VC):
    import concourse.bacc as bacc
    import concourse.bass as bass
    import concourse.mybir as mybir
    import concourse.tile as tile
    from concourse.bass import IndirectOffsetOnAxis
    from concourse.masks import make_identity

    f32 = mybir.dt.float32
    bf16 = mybir.dt.bfloat16
    i32 = mybir.dt.int32
    Alu = mybir.AluOpType
    AX = mybir.AxisListType

    nc = bacc.Bacc(
        "TRN2",
        target_bir_lowering=False,
        debug=False,
        num_devices=NCORES,
    )

    m_in = nc.dram_tensor("m_rw", [S, S], bf16, kind="ExternalInput").ap()
    ntmT_in = nc.dram_tensor("ntmT", [S, 256], bf16, kind="ExternalInput").ap()
    codes_in = nc.dram_tensor("codes", [P, 4], i32, kind="ExternalInput").ap()
    emb_in = nc.dram_tensor("emb_table", [V, D], f32, kind="ExternalInput").ap()
    w_in = nc.dram_tensor("w_pad", [DPAD, vc], bf16, kind="ExternalInput").ap()
    logit_out = nc.dram_tensor("logit", [NTOK, vc], bf16, kind="ExternalOutput").ap()

    n_full, n_rem = divmod(vc, 512)
    ntiles = [512] * n_full + ([n_rem] if n_rem else [])

    with tile.TileContext(nc) as tc:
        with (
            tc.tile_pool(name="persist", bufs=1) as persist,
            tc.tile_pool(name="dram", bufs=1, space="DRAM") as dram,
        ):
            xTg_a1 = persist.tile([P, NCORES * 3, 256], bf16, name="xTg_a1")
            xTg_a2 = persist.tile([P, NCORES * 3, 256], bf16, name="xTg_a2")
            xTg_b = persist.tile([P, NCORES, 256], bf16, name="xTg_b")
            xT_a = persist.tile([P, 6, 256], bf16, name="xT_a")
            xT_b = persist.tile([P, 1, 256], bf16, name="xT_b")
            w_all = persist.tile([P, KCH, vc], bf16, name="w_all")
            ident = persist.tile([P, P], f32, name="ident")

            cc_in_a1 = dram.tile([3 * P, 256], bf16, name="cc_in_a1")
            cc_out_a1 = dram.tile(
                [NCORES * 3 * P, 256], bf16, name="cc_out_a1", addr_space="Shared"
            )
            cc_in_a2 = dram.tile([3 * P, 256], bf16, name="cc_in_a2")
            cc_out_a2 = dram.tile(
                [NCORES * 3 * P, 256], bf16, name="cc_out_a2", addr_space="Shared"
            )
            cc_in_b = dram.tile([P, 256], bf16, name="cc_in_b")
            cc_out_b = dram.tile(
                [NCORES * P, 256], bf16, name="cc_out_b", addr_space="Shared"
            )

            # ---------------- Stage 1 ----------------
            with (
                tc.tile_pool(name="s1", bufs=1) as s1,
                tc.tile_pool(name="s1tmp", bufs=3) as s1t,
                tc.tile_pool(name="psq", bufs=3, space="PSUM") as psq,
                tc.tile_pool(name="psa", bufs=3, space="PSUM") as psa,
                tc.tile_pool(name="pst", bufs=2, space="PSUM") as pst,
            ):
                codes_sb = s1.tile([P, 4], i32, name="codes_sb")
                nc.sync.dma_start(out=codes_sb[:], in_=codes_in)
                ntmT_sb = s1.tile([P, 4, 256], bf16, name="ntmT_sb")
                nc.sync.dma_start(
                    out=ntmT_sb[:], in_=ntmT_in.rearrange("(j p) r -> p j r", p=P)
                )
                m_sb = s1.tile([P, 4, S], bf16, name="m_sb")
                nc.sync.dma_start(
                    out=m_sb[:], in_=m_in.rearrange("(j p) s -> p j s", p=P)
                )
                # w resident: needed from ~105us; small stage-1 inputs first
                w_re = w_in.rearrange("(k p) v -> p k v", p=P)
                for k in range(KCH):
                    nc.sync.dma_start(out=w_all[:, k, :], in_=w_re[:, k, :])

                # eye blocks for Q0 init + diag extraction mask
                q0f = s1.tile([P, 4, 256], f32, name="q0f")
                nc.gpsimd.memset(q0f[:], 0.0)
                for jb in range(2):
                    nc.gpsimd.affine_select(
                        out=q0f[:, jb, :],
                        in_=q0f[:, jb, :],
                        compare_op=Alu.not_equal,
                        fill=1.0,
                        base=jb * P,
                        pattern=[[-1, 256]],
                        channel_multiplier=1,
                    )

                pe_pad = s1.tile([P, 2, P], f32, name="pe_pad")
                nc.gpsimd.memset(pe_pad[:], 0.0)
                nc.vector.memset(pe_pad[:, :, KPE : KPE + 1], 1.0)

                # gather embeddings for all 512 tokens of this core's batch
                emb_sb = s1.tile([P, 4, D], f32, name="emb_sb")
                for k in range(4):
                    nc.gpsimd.indirect_dma_start(
                        out=emb_sb[:, k, :],
                        out_offset=None,
                        in_=emb_in[:, :],
                        in_offset=IndirectOffsetOnAxis(
                            ap=codes_sb[:, k : k + 1], axis=0
                        ),
                    )
                make_identity(nc, ident[:])

                # ---- RWPE chain FIRST on the PE (CC boot ~67us anyway) ----
                qA = s1.tile([P, 4, 256], bf16, name="qA")
                qB = s1.tile([P, 4, 256], bf16, name="qB")
                nc.vector.tensor_copy(out=qA[:], in_=q0f[:])

                cur = qA
                for t in range(KPE):
                    nxt = qB if cur is qA else qA
                    for i in range(4):
                        pq = psq.tile([P, 256], f32, tag="pq")
                        for j in range(4):
                            nc.tensor.matmul(
                                out=pq[:],
                                lhsT=m_sb[:, j, i * P : (i + 1) * P],
                                rhs=cur[:, j, :],
                                start=(j == 0),
                                stop=(j == 3),
                            )
                        if i < 2:
                            dummy = s1t.tile([P, 256], f32, tag="ttr_dummy")
                            nc.vector.tensor_mul(dummy[:], pq[:], q0f[:, i, 0:256])
                            nc.vector.reduce_sum(
                                out=pe_pad[:, i, t : t + 1], in_=dummy[:], axis=AX.X
                            )
                        if i == 0:
                            nc.vector.tensor_copy(out=nxt[:, i, :], in_=pq[:])
                        else:
                            nc.scalar.copy(out=nxt[:, i, :], in_=pq[:])
                    cur = nxt

                # pe chunk: transpose [tokens, pe] -> [pe, tokens]
                for lj in range(2):
                    pt = pst.tile([P, P], f32, tag="pt")
                    nc.tensor.transpose(
                        out=pt[:], in_=pe_pad[:, lj, :], identity=ident[:]
                    )
                    nc.vector.tensor_copy(
                        out=xT_b[:, 0, lj * P : (lj + 1) * P], in_=pt[:]
                    )
                nc.sync.dma_start(out=cc_in_b[:, :], in_=xT_b[:, 0, :])

                # ---- emb cast + xT emb chunks (after chain on PE) ----
                emb_bf = s1.tile([P, 4, D], bf16, name="emb_bf")
                for k in range(4):
                    if k % 2 == 0:
                        nc.scalar.copy(out=emb_bf[:, k, :], in_=emb_sb[:, k, :])
                    else:
                        nc.vector.tensor_copy(out=emb_bf[:, k, :], in_=emb_sb[:, k, :])

                for w0 in (0, 3):
                    pas = {}
                    for j in range(4):
                        for c in range(w0, w0 + 3):
                            if j == 0:
                                pas[c] = psa.tile(
                                    [P, 256], f32, tag="pa", name=f"pa{c}"
                                )
                            nc.tensor.matmul(
                                out=pas[c][:],
                                lhsT=emb_bf[:, j, c * P : (c + 1) * P],
                                rhs=ntmT_sb[:, j, :],
                                start=(j == 0),
                                stop=(j == 3),
                            )
                    for c in range(w0, w0 + 3):
                        if c % 2 == 0:
                            nc.scalar.copy(out=xT_a[:, c, :], in_=pas[c][:])
                        else:
                            nc.vector.tensor_copy(out=xT_a[:, c, :], in_=pas[c][:])
                nc.sync.dma_start(
                    out=cc_in_a1.rearrange("(k p) r -> p k r", p=P),
                    in_=xT_a[:, 0:3, :],
                )
                nc.sync.dma_start(
                    out=cc_in_a2.rearrange("(k p) r -> p k r", p=P),
                    in_=xT_a[:, 3:6, :],
                )

                # ---- AllGathers (A split in two so the first half lands
                # ~10us earlier; B = pe chunk last) ----
                nc.gpsimd.collective_compute(
                    "AllGather",
                    mybir.AluOpType.bypass,
                    replica_groups=[list(range(NCORES))],
                    ins=[cc_in_a1[:].opt()],
                    outs=[cc_out_a1[:].opt()],
                )
                nc.gpsimd.collective_compute(
                    "AllGather",
                    mybir.AluOpType.bypass,
                    replica_groups=[list(range(NCORES))],
                    ins=[cc_in_a2[:].opt()],
                    outs=[cc_out_a2[:].opt()],
                )
                nc.gpsimd.collective_compute(
                    "AllGather",
                    mybir.AluOpType.bypass,
                    replica_groups=[list(range(NCORES))],
                    ins=[cc_in_b[:].opt()],
                    outs=[cc_out_b[:].opt()],
                )
                cc_re1 = cc_out_a1.rearrange("(ck p) r -> p ck r", p=P)
                cc_re2 = cc_out_a2.rearrange("(ck p) r -> p ck r", p=P)
                for sl in range(8):
                    nc.sync.dma_start(
                        out=xTg_a1[:, sl * 3 : (sl + 1) * 3, :],
                        in_=cc_re1[:, sl * 3 : (sl + 1) * 3, :],
                    )
                for sl in range(8):
                    nc.sync.dma_start(
                        out=xTg_a2[:, sl * 3 : (sl + 1) * 3, :],
                        in_=cc_re2[:, sl * 3 : (sl + 1) * 3, :],
                    )
                nc.sync.dma_start(
                    out=xTg_b[:],
                    in_=cc_out_b.rearrange("(c p) r -> p c r", p=P),
                )

            # ---------------- Stage 2: logits = xT.T @ w ----------------
            with (
                tc.tile_pool(name="ob", bufs=2) as obp,
                tc.tile_pool(name="ps2", bufs=8, space="PSUM") as ps2,
            ):
                def lhs_for(rc, lj, k):
                    if k < 3:
                        return xTg_a1[:, rc * 3 + k, lj * P : (lj + 1) * P]
                    if k < 6:
                        return xTg_a2[:, rc * 3 + (k - 3), lj * P : (lj + 1) * P]
                    return xTg_b[:, rc, lj * P : (lj + 1) * P]

                # pre-phase: m=0's first 8 n-tiles accumulate k0..5 into all 8
                # PSUM banks while the pe-chunk AllGather is still in flight
                held = {}
                for n in range(8):
                    po = ps2.tile([P, 512], f32, tag="po")
                    for k in range(3):
                        nc.tensor.matmul(
                            out=po[:],
                            lhsT=lhs_for(0, 0, k),
                            rhs=w_all[:, k, n * 512 : (n + 1) * 512],
                            start=(k == 0),
                            stop=False,
                        )
                    held[n] = po
                for n in range(8):
                    for k in range(3, 6):
                        nc.tensor.matmul(
                            out=held[n][:],
                            lhsT=lhs_for(0, 0, k),
                            rhs=w_all[:, k, n * 512 : (n + 1) * 512],
                            start=False,
                            stop=False,
                        )

                cuts = {4 * 512: (0, 4 * 512), 8 * 512: (4 * 512, 8 * 512),
                        12 * 512: (8 * 512, 12 * 512)}
                for m in range(MT):
                    rc, lj = divmod(m, 2)
                    ob = obp.tile([P, vc], bf16, tag="ob")
                    for n, ntn in enumerate(ntiles):
                        n0 = n * 512
                        if m == 0 and n < 8:
                            po = held[n]
                            nc.tensor.matmul(
                                out=po[:, 0:ntn],
                                lhsT=lhs_for(rc, lj, 6),
                                rhs=w_all[:, 6, n0 : n0 + ntn],
                                start=False,
                                stop=True,
                            )
                        else:
                            po = ps2.tile([P, 512], f32, tag="po")
                            for k in range(KCH):
                                nc.tensor.matmul(
                                    out=po[:, 0:ntn],
                                    lhsT=lhs_for(rc, lj, k),
                                    rhs=w_all[:, k, n0 : n0 + ntn],
                                    start=(k == 0),
                                    stop=(k == KCH - 1),
                                )
                        if n % 2 == 0:
                            nc.scalar.copy(out=ob[:, n0 : n0 + ntn], in_=po[:, 0:ntn])
                        else:
                            nc.vector.tensor_copy(
                                out=ob[:, n0 : n0 + ntn], in_=po[:, 0:ntn]
                            )
                        if n0 + ntn in cuts:
                            lo, hi = cuts[n0 + ntn]
                            nc.sync.dma_start(
                                out=logit_out[m * P : (m + 1) * P, lo:hi],
                                in_=ob[:, lo:hi],
                            )
                    nc.sync.dma_start(
                        out=logit_out[m * P : (m + 1) * P, 12 * 512 : vc],
                        in_=ob[:, 12 * 512 : vc],
                    )

    nc.compile()
    return nc


def _host_prep(code_inputs, position_idx, attn_mask, emb_table, w_lin, b_lin, vc=VC):
    import ml_dtypes

    bf = ml_dtypes.bfloat16
    code = np.asarray(code_inputs).astype(np.int32)
    pos = np.asarray(position_idx).astype(np.int32)
    attn = np.asarray(attn_mask).astype(np.float32)
    emb_t = np.ascontiguousarray(np.asarray(emb_table, dtype=np.float32))
    w = np.asarray(w_lin, dtype=np.float32)
    bias = np.asarray(b_lin, dtype=np.float32)

    w_ext = np.zeros((DPAD, NCORES * vc), np.float32)
    ncols = min(NCORES * vc, V)
    w_ext[: D + KPE, :ncols] = w[:, :ncols]
    w_ext[D + KPE, :ncols] = bias[:ncols]
    w_ext = w_ext.astype(bf)

    nodes = (pos == 0).astype(np.float32)
    token = (pos >= 2).astype(np.float32)
    eye = np.eye(S, dtype=bool)

    in_maps = []
    for c in range(NCORES):
        b, h = divmod(c, 2)
        if h == 0:
            perm = np.arange(S)
        else:
            perm = np.r_[256:512, 0:256]
        a_p = attn[b][perm][:, perm]
        tok_p = token[b][perm]
        nod_p = nodes[b][perm]

        A = np.where(eye, 1.0, a_p).astype(np.float32)
        m_rw = A / A.sum(1)[:, None]

        rowsum = (a_p[:256] * tok_p[None, :]).sum(1)
        alpha = nod_p[:256] / (rowsum + 1e-10)
        ntmT = a_p[:256].T * tok_p[:, None] * alpha[None, :]
        ntmT[:256][np.eye(256, dtype=bool)] += 1.0 - nod_p[:256]

        in_maps.append(
            {
                "m_rw": np.ascontiguousarray(m_rw.astype(bf)),
                "ntmT": np.ascontiguousarray(ntmT.astype(bf)),
                "codes": np.ascontiguousarray(code[b][perm].reshape(4, P).T),
                "emb_table": emb_t,
                "w_pad": np.ascontiguousarray(w_ext[:, c * vc : (c + 1) * vc]),
            }
        )
    return in_maps


def run(inputs, trace=False, vc=VC, **run_kwargs):
    from concourse.bass_utils import run_bass_kernel_spmd

    key = ("nc", vc)
    nc = _CACHE.get(key)
    if nc is None:
        nc = _build_nc(vc=vc)
        _CACHE[key] = nc
    in_maps = _host_prep(**inputs, vc=vc)
    res = run_bass_kernel_spmd(
        nc, in_maps, core_ids=list(range(NCORES)), trace=trace, **run_kwargs
    )
    ncols = min(NCORES * vc, V)
    logits = np.concatenate(
        [r["logit"].astype(np.float32) for r in res.results], axis=1
    )[:, :ncols]
    return logits.reshape(B, S, ncols).astype(np.float32), res


def kernel(**inputs):
    logits, _ = run(inputs, trace=False)
    return logits


# revision 20
# speedup vs baseline: 1.1473x; 1.1473x over previous
# Trainium2 Bass kernel for nn_ModelPositional (gnn_message_passing).
# v8: chain-first ordering — the collective firmware needs ~67us to boot,
# so run the RWPE chain first on the PE, land both AllGather payloads
# just as the CC comes up, and bridge the remaining gather latency with
# the stage-2 pre-phase (k0..5 held across all 8 PSUM banks).

import numpy as np

B, S, KPE, V, D = 4, 512, 16, 50265, 768
NCORES = 8
VPAD = 50272          # 8 * 6284
VC = VPAD // NCORES   # 6284 vocab columns per core
DPAD = 896            # 7 * 128 (784 true dims + bias row at 784 + zero pad)
KCH = DPAD // 128     # 7 contraction chunks
P = 128
NTOK = B * S          # 2048
MT = NTOK // P        # 16 token tiles

_CACHE = {}


def _build_nc(vc=VC):
    import concourse.bacc as bacc
    import concourse.bass as bass
    import concourse.mybir as mybir
    import concourse.tile as tile
    from concourse.bass import IndirectOffsetOnAxis
    from concourse.masks import make_identity

    f32 = mybir.dt.float32
    bf16 = mybir.dt.bfloat16
    i32 = mybir.dt.int32
    Alu = mybir.AluOpType
    AX = mybir.AxisListType

    nc = bacc.Bacc(
        "TRN2",
        target_bir_lowering=False,
        debug=False,
        num_devices=NCORES,
    )

    m_in = nc.dram_tensor("m_rw", [S, S], bf16, kind="ExternalInput").ap()
    ntmT_in = nc.dram_tensor("ntmT", [S, 256], bf16, kind="ExternalInput").ap()
    codes_in = nc.dram_tensor("codes", [P, 4], i32, kind="ExternalInput").ap()
    emb_in = nc.dram_tensor("emb_table", [V, D], f32, kind="ExternalInput").ap()
    w_in = nc.dram_tensor("w_pad", [DPAD, vc], bf16, kind="ExternalInput").ap()
    logit_out = nc.dram_tensor("logit", [NTOK, vc], bf16, kind="ExternalOutput").ap()

    n_full, n_rem = divmod(vc, 512)
    ntiles = [512] * n_full + ([n_rem] if n_rem else [])

    with tile.TileContext(nc) as tc:
        with (
            tc.tile_pool(name="persist", bufs=1) as persist,
            tc.tile_pool(name="dram", bufs=1, space="DRAM") as dram,
        ):
            xTg_a = persist.tile([P, NCORES * 6, 256], bf16, name="xTg_a")
            xTg_b = persist.tile([P, NCORES, 256], bf16, name="xTg_b")
            xT_a = persist.tile([P, 6, 256], bf16, name="xT_a")
            xT_b = persist.tile([P, 1, 256], bf16, name="xT_b")
            w_all = persist.tile([P, KCH, vc], bf16, name="w_all")
            ident = persist.tile([P, P], f32, name="ident")

            cc_in_a = dram.tile([6 * P, 256], bf16, name="cc_in_a")
            cc_out_a = dram.tile(
                [NCORES * 6 * P, 256], bf16, name="cc_out_a", addr_space="Shared"
            )
            cc_in_b = dram.tile([P, 256], bf16, name="cc_in_b")
            cc_out_b = dram.tile(
                [NCORES * P, 256], bf16, name="cc_out_b", addr_space="Shared"
            )

            # ---------------- Stage 1 ----------------
            with (
                tc.tile_pool(name="s1", bufs=1) as s1,
                tc.tile_pool(name="s1tmp", bufs=3) as s1t,
                tc.tile_pool(name="psq", bufs=3, space="PSUM") as psq,
                tc.tile_pool(name="psa", bufs=3, space="PSUM") as psa,
                tc.tile_pool(name="pst", bufs=2, space="PSUM") as pst,
            ):
                codes_sb = s1.tile([P, 4], i32, name="codes_sb")
                nc.sync.dma_start(out=codes_sb[:], in_=codes_in)
                actw = s1.tile([P, 2], f32, name="actw")
                nc.vector.memset(actw[:, 0:1], 0.0)
                nc.scalar.copy(out=actw[:, 1:2], in_=actw[:, 0:1])
                ntmT_sb = s1.tile([P, 4, 256], bf16, name="ntmT_sb")
                nc.sync.dma_start(
                    out=ntmT_sb[:], in_=ntmT_in.rearrange("(j p) r -> p j r", p=P)
                )
                m_sb = s1.tile([P, 4, S], bf16, name="m_sb")
                nc.sync.dma_start(
                    out=m_sb[:], in_=m_in.rearrange("(j p) s -> p j s", p=P)
                )
                # w resident: needed from ~105us; small stage-1 inputs first
                w_re = w_in.rearrange("(k p) v -> p k v", p=P)
                for k in range(KCH):
                    nc.sync.dma_start(out=w_all[:, k, :], in_=w_re[:, k, :])

                # eye blocks for Q0 init + diag extraction mask
                q0f = s1.tile([P, 4, 256], f32, name="q0f")
                nc.gpsimd.memset(q0f[:], 0.0)
                for jb in range(2):
                    nc.gpsimd.affine_select(
                        out=q0f[:, jb, :],
                        in_=q0f[:, jb, :],
                        compare_op=Alu.not_equal,
                        fill=1.0,
                        base=jb * P,
                        pattern=[[-1, 256]],
                        channel_multiplier=1,
                    )

                pe_pad = s1.tile([P, 2, P], f32, name="pe_pad")
                nc.gpsimd.memset(pe_pad[:], 0.0)
                nc.vector.memset(pe_pad[:, :, KPE : KPE + 1], 1.0)

                # gather embeddings for all 512 tokens of this core's batch
                emb_sb = s1.tile([P, 4, D], f32, name="emb_sb")
                for k in range(4):
                    nc.gpsimd.indirect_dma_start(
                        out=emb_sb[:, k, :],
                        out_offset=None,
                        in_=emb_in[:, :],
                        in_offset=IndirectOffsetOnAxis(
                            ap=codes_sb[:, k : k + 1], axis=0
                        ),
                    )
                make_identity(nc, ident[:])

                # ---- RWPE chain FIRST on the PE (CC boot ~67us anyway) ----
                qA = s1.tile([P, 4, 256], bf16, name="qA")
                qB = s1.tile([P, 4, 256], bf16, name="qB")
                nc.vector.tensor_copy(out=qA[:], in_=q0f[:])

                cur = qA
                for t in range(KPE):
                    nxt = qB if cur is qA else qA
                    for i in range(4):
                        pq = psq.tile([P, 256], f32, tag="pq")
                        for j in range(4):
                            nc.tensor.matmul(
                                out=pq[:],
                                lhsT=m_sb[:, j, i * P : (i + 1) * P],
                                rhs=cur[:, j, :],
                                start=(j == 0),
                                stop=(j == 3),
                            )
                        if i < 2:
                            dummy = s1t.tile([P, 256], f32, tag="ttr_dummy")
                            nc.vector.tensor_mul(dummy[:], pq[:], q0f[:, i, 0:256])
                            nc.vector.reduce_sum(
                                out=pe_pad[:, i, t : t + 1], in_=dummy[:], axis=AX.X
                            )
                        if i == 0:
                            nc.vector.tensor_copy(out=nxt[:, i, :], in_=pq[:])
                        else:
                            nc.scalar.copy(out=nxt[:, i, :], in_=pq[:])
                    cur = nxt

                # pe chunk: transpose [tokens, pe] -> [pe, tokens]
                for lj in range(2):
                    pt = pst.tile([P, P], f32, tag="pt")
                    nc.tensor.transpose(
                        out=pt[:], in_=pe_pad[:, lj, :], identity=ident[:]
                    )
                    nc.vector.tensor_copy(
                        out=xT_b[:, 0, lj * P : (lj + 1) * P], in_=pt[:]
                    )
                nc.sync.dma_start(out=cc_in_b[:, :], in_=xT_b[:, 0, :])

                # ---- emb cast + xT emb chunks (after chain on PE) ----
                emb_bf = s1.tile([P, 4, D], bf16, name="emb_bf")
                for k in range(4):
                    if k % 2 == 0:
                        nc.scalar.copy(out=emb_bf[:, k, :], in_=emb_sb[:, k, :])
                    else:
                        nc.vector.tensor_copy(out=emb_bf[:, k, :], in_=emb_sb[:, k, :])

                for w0 in (0, 3):
                    pas = {}
                    for j in range(4):
                        for c in range(w0, w0 + 3):
                            if j == 0:
                                pas[c] = psa.tile(
                                    [P, 256], f32, tag="pa", name=f"pa{c}"
                                )
                            nc.tensor.matmul(
                                out=pas[c][:],
                                lhsT=emb_bf[:, j, c * P : (c + 1) * P],
                                rhs=ntmT_sb[:, j, :],
                                start=(j == 0),
                                stop=(j == 3),
                            )
                    for c in range(w0, w0 + 3):
                        if c % 2 == 0:
                            nc.scalar.copy(out=xT_a[:, c, :], in_=pas[c][:])
                        else:
                            nc.vector.tensor_copy(out=xT_a[:, c, :], in_=pas[c][:])
                nc.sync.dma_start(
                    out=cc_in_a.rearrange("(k p) r -> p k r", p=P), in_=xT_a[:]
                )

                # ---- AllGathers (A = emb chunks first, B = pe chunk) ----
                nc.gpsimd.collective_compute(
                    "AllGather",
                    mybir.AluOpType.bypass,
                    replica_groups=[list(range(NCORES))],
                    ins=[cc_in_a[:].opt()],
                    outs=[cc_out_a[:].opt()],
                )
                nc.gpsimd.collective_compute(
                    "AllGather",
                    mybir.AluOpType.bypass,
                    replica_groups=[list(range(NCORES))],
                    ins=[cc_in_b[:].opt()],
                    outs=[cc_out_b[:].opt()],
                )
                cc_re = cc_out_a.rearrange("(ck p) r -> p ck r", p=P)
                for sl in range(8):
                    nc.sync.dma_start(
                        out=xTg_a[:, sl * 6 : (sl + 1) * 6, :],
                        in_=cc_re[:, sl * 6 : (sl + 1) * 6, :],
                    )
                nc.sync.dma_start(
                    out=xTg_b[:],
                    in_=cc_out_b.rearrange("(c p) r -> p c r", p=P),
                )

            # ---------------- Stage 2: logits = xT.T @ w ----------------
            with (
                tc.tile_pool(name="ob", bufs=2) as obp,
                tc.tile_pool(name="ps2", bufs=8, space="PSUM") as ps2,
            ):
                def lhs_for(rc, lj, k):
                    if k < 6:
                        return xTg_a[:, rc * 6 + k, lj * P : (lj + 1) * P]
                    return xTg_b[:, rc, lj * P : (lj + 1) * P]

                # pre-phase: m=0's first 8 n-tiles accumulate k0..5 into all 8
                # PSUM banks while the pe-chunk AllGather is still in flight
                held = {}
                for n in range(8):
                    po = ps2.tile([P, 512], f32, tag="po")
                    for k in range(6):
                        nc.tensor.matmul(
                            out=po[:],
                            lhsT=lhs_for(0, 0, k),
                            rhs=w_all[:, k, n * 512 : (n + 1) * 512],
                            start=(k == 0),
                            stop=False,
                        )
                    held[n] = po

                cuts = {4 * 512: (0, 4 * 512), 8 * 512: (4 * 512, 8 * 512),
                        12 * 512: (8 * 512, 12 * 512)}
                for m in range(MT):
                    rc, lj = divmod(m, 2)
                    ob = obp.tile([P, vc], bf16, tag="ob")
                    for n, ntn in enumerate(ntiles):
                        n0 = n * 512
                        if m == 0 and n < 8:
                            po = held[n]
                            nc.tensor.matmul(
                                out=po[:, 0:ntn],
                                lhsT=lhs_for(rc, lj, 6),
                                rhs=w_all[:, 6, n0 : n0 + ntn],
                                start=False,
                                stop=True,
                            )
                        else:
                            po = ps2.tile([P, 512], f32, tag="po")
                            for k in range(KCH):
                                nc.tensor.matmul(
                                    out=po[:, 0:ntn],
                                    lhsT=lhs_for(rc, lj, k),
                                    rhs=w_all[:, k, n0 : n0 + ntn],
                                    start=(k == 0),
                                    stop=(k == KCH - 1),
                                )
                        if n % 2 == 0:
                            nc.scalar.copy(out=ob[:, n0 : n0 + ntn], in_=po[:, 0:ntn])
                        else:
                            nc.vector.tensor_copy(
                                out=ob[:, n0 : n0 + ntn], in_=po[:, 0:ntn]
                            )
                        if n0 + ntn in cuts:
                            lo, hi = cuts[n0 + ntn]
                            nc.sync.dma_start(
                                out=logit_out[m * P : (m + 1) * P, lo:hi],
                                in_=ob[:, lo:hi],
                            )
                    nc.sync.dma_start(
                        out=logit_out[m * P : (m + 1) * P, 12 * 512 : vc],
                        in_=ob[:, 12 * 512 : vc],
                    )

    nc.compile()
    return nc


def _host_prep(code_inputs, position_idx, attn_mask, emb_table, w_lin, b_lin, vc=VC):
    import ml_dtypes

    bf = ml_dtypes.bfloat16
    code = np.asarray(code_inputs).astype(np.int32)
    pos = np.asarray(position_idx).astype(np.int32)
    attn = np.asarray(attn_mask).astype(np.float32)
    emb_t = np.ascontiguousarray(np.asarray(emb_table, dtype=np.float32))
    w = np.asarray(w_lin, dtype=np.float32)
    bias = np.asarray(b_lin, dtype=np.float32)

    w_ext = np.zeros((DPAD, NCORES * vc), np.float32)
    ncols = min(NCORES * vc, V)
    w_ext[: D + KPE, :ncols] = w[:, :ncols]
    w_ext[D + KPE, :ncols] = bias[:ncols]
    w_ext = w_ext.astype(bf)

    nodes = (pos == 0).astype(np.float32)
    token = (pos >= 2).astype(np.float32)
    eye = np.eye(S, dtype=bool)

    in_maps = []
    for c in range(NCORES):
        b, h = divmod(c, 2)
        if h == 0:
            perm = np.arange(S)
        else:
            perm = np.r_[256:512, 0:256]
        a_p = attn[b][perm][:, perm]
        tok_p = token[b][perm]
        nod_p = nodes[b][perm]

        A = np.where(eye, 1.0, a_p).astype(np.float32)
        m_rw = A / A.sum(1)[:, None]

        rowsum = (a_p[:256] * tok_p[None, :]).sum(1)
        alpha = nod_p[:256] / (rowsum + 1e-10)
        ntmT = a_p[:256].T * tok_p[:, None] * alpha[None, :]
        ntmT[:256][np.eye(256, dtype=bool)] += 1.0 - nod_p[:256]

        in_maps.append(
            {
                "m_rw": np.ascontiguousarray(m_rw.astype(bf)),
                "ntmT": np.ascontiguousarray(ntmT.astype(bf)),
                "codes": np.ascontiguousarray(code[b][perm].reshape(4, P).T),
                "emb_table": emb_t,
                "w_pad": np.ascontiguousarray(w_ext[:, c * vc : (c + 1) * vc]),
            }
        )
    return in_maps


def run(inputs, trace=False, vc=VC, **run_kwargs):
    from concourse.bass_utils import run_bass_kernel_spmd

    key = ("nc", vc)
    nc = _CACHE.get(key)
    if nc is None:
        nc = _build_nc(vc=vc)
        _CACHE[key] = nc
    in_maps = _host_prep(**inputs, vc=vc)
    res = run_bass_kernel_spmd(
        nc, in_maps, core_ids=list(range(NCORES)), trace=trace, **run_kwargs
    )
    ncols = min(NCORES * vc, V)
    logits = np.concatenate(
        [r["logit"].astype(np.float32) for r in res.results], axis=1
    )[:, :ncols]
    return logits.reshape(B, S, ncols).astype(np.float32), res


def kernel(**inputs):
    logits, _ = run(inputs, trace=False)
    return logits
